# revision 5
# baseline (speedup 1.0000x reference)
"""Data-parallel CorrelationalDetector kernel for 8 Trainium2 NeuronCores.

Strategy (per spec sharding_hint): pure data parallel over the batch dim,
weights replicated. The end-to-end call is transfer-bound (the axon tunnel
moves ~50 MB/s and the raw inputs are ~53 MB), so the kernel is split to
minimize bytes shipped to the device:

  1. Layer 0 (stride-2 conv, 3->3ch) runs on the host in fp32. Its output
     is 4x smaller than the raw frame (12.6 MB + 0.8 MB vs 53 MB).
  2. Layers 1..4 + the per-sample cross-correlation run on the 8 NeuronCores
     (batch sharded 8 ways, 8 samples/core). The conv weights W1..W4 are
     baked into the compiled executable as constants (shipped once at
     compile time, not per call; a byte-exact guard recompiles if the
     weights ever change).
  3. BatchNorm batch stats are reduced on the host from the gathered
     pre-BN response maps (the gather IS the all-reduce; exact global
     stats), then normalization is applied on the host (40 KB of work).

Inputs are fp32 end to end: the correctness gate (max rel err vs a fp32
reference with denom max(|e|,1e-3)) leaves no room for bf16/fp16 transport.
"""

import hashlib

import numpy as np
import jax
import jax.numpy as jnp
from jax import lax

_DN = ("NCHW", "OIHW", "NCHW")
_N_CORES = 8

# (out_ch, kernel, stride) for layers 1..4 (layer 0 is done on the host).
_DEV_STRIDES = (2, 1, 2, 1)

_cache = {"wkey": None, "fn": None}


def _conv0_host(x, W, b):
    """Stride-2 3x3 conv, pad 1, + bias + relu, in fp32 on the host.

    x: [B,C,H,W] -> [B,O,H//2,W//2]
    """
    Bn, C, H, Wd = x.shape
    O = W.shape[0]
    Ho, Wo = H // 2, Wd // 2
    xp = np.zeros((Bn, C, H + 2, Wd + 2), np.float32)
    xp[:, :, 1:-1, 1:-1] = x
    N = Bn * Ho * Wo
    col = np.empty((C * 9, N), np.float32)
    k = 0
    for c in range(C):
        for dy in range(3):
            for dx in range(3):
                np.copyto(
                    col[k].reshape(Bn, Ho, Wo), xp[:, c, dy : dy + H : 2, dx : dx + Wd : 2]
                )
                k += 1
    out = W.reshape(O, C * 9) @ col  # [O, N]
    out += b[:, None]
    np.maximum(out, 0, out=out)
    return np.ascontiguousarray(out.reshape(O, Bn, Ho, Wo).transpose(1, 0, 2, 3))


def _build_fn(Ws, bs):
    """pmap'd L1..L4 + xcorr with the weights closed over as constants."""
    Wj = [jnp.asarray(W) for W in Ws]
    bj = [jnp.asarray(b) for b in bs]

    def shard_fn(f0, c0):
        def enc(x):
            for i, s in enumerate(_DEV_STRIDES):
                x = lax.conv_general_dilated(
                    x, Wj[i], (s, s), ((1, 1), (1, 1)), dimension_numbers=_DN
                )
                x = x + bj[i][None, :, None, None]
                if i < 3:
                    x = jax.nn.relu(x)
            return x

        frame_fm = enc(f0)  # [bl,256,32,32]
        crop_fm = enc(c0)   # [bl,256,8,8]

        def xcorr(f, k):
            return lax.conv_general_dilated(
                f[None], k[None], (1, 1), "VALID", dimension_numbers=_DN
            )[0]

        rmap = jax.vmap(xcorr)(frame_fm, crop_fm)  # [bl,1,25,25]
        # On-device gather across the 8 cores (fast on-chip links) so the
        # host fetches the full result from a single device instead of
        # paying 8 serialized tunnel round-trips.
        return lax.all_gather(rmap, "x")  # [8,bl,1,25,25] on every core

    return jax.pmap(shard_fn, axis_name="x", devices=jax.devices()[:_N_CORES])


def kernel(crop, frame, W0, b0, W1, b1, W2, b2, W3, b3, W4, b4, gamma, beta):
    crop = np.asarray(crop, np.float32)
    frame = np.asarray(frame, np.float32)
    W0 = np.asarray(W0, np.float32)
    b0 = np.asarray(b0, np.float32)
    Ws = [np.asarray(w, np.float32) for w in (W1, W2, W3, W4)]
    bs = [np.asarray(b, np.float32) for b in (b1, b2, b3, b4)]

    # Host layer 0 (the transfer-size reducer).
    f0 = _conv0_host(frame, W0, b0)  # [64,3,128,128]
    c0 = _conv0_host(crop, W0, b0)   # [64,3,32,32]

    B = f0.shape[0]
    bl = B // _N_CORES
    f0_sh = f0.reshape(_N_CORES, bl, *f0.shape[1:])
    c0_sh = c0.reshape(_N_CORES, bl, *c0.shape[1:])

    # Weight-baked executable, rebuilt only if the weight bytes change.
    h = hashlib.blake2b(digest_size=16)
    for a in (*Ws, *bs):
        h.update(a.tobytes())
    wkey = h.hexdigest()
    if _cache["wkey"] != wkey:
        _cache["fn"] = _build_fn(Ws, bs)
        _cache["wkey"] = wkey

    rmap_sh = _cache["fn"](f0_sh, c0_sh)
    # Every device holds the gathered [8,bl,1,25,25]; read device 0 only.
    rmap = np.asarray(rmap_sh[0]).reshape(B, 1, 25, 25).astype(np.float64)

    # Global BatchNorm2d(1), training mode; exact batch stats on the host.
    mean = rmap.mean()
    var = rmap.var()
    g = np.asarray(gamma, np.float32).reshape(1, -1, 1, 1)
    bt = np.asarray(beta, np.float32).reshape(1, -1, 1, 1)
    out = (rmap - mean) / np.sqrt(var + 1e-5)
    out = out.astype(np.float32) * g + bt
    return out.astype(np.float32)


# revision 7
# speedup vs baseline: 1.0653x; 1.0653x over previous
"""Data-parallel CorrelationalDetector kernel for 8 Trainium2 NeuronCores.

Strategy (per spec sharding_hint): pure data parallel over the batch dim,
weights replicated. The end-to-end call is transfer-bound (the axon tunnel
moves ~50 MB/s and the raw inputs are ~53 MB), so the kernel is split to
minimize bytes shipped to the device:

  1. Layer 0 (stride-2 conv, 3->3ch) runs on the host in fp32. Its output
     is 4x smaller than the raw frame (12.6 MB + 0.8 MB vs 53 MB).
  2. Layers 1..4 + the per-sample cross-correlation run on the 8 NeuronCores
     (batch sharded 8 ways, 8 samples/core). The conv weights W1..W4 are
     baked into the compiled executable as constants (shipped once at
     compile time, not per call; a byte-exact guard recompiles if the
     weights ever change).
  3. BatchNorm batch stats are reduced on the host from the gathered
     pre-BN response maps (the gather IS the all-reduce; exact global
     stats), then normalization is applied on the host (40 KB of work).

Inputs are fp32 end to end: the correctness gate (max rel err vs a fp32
reference with denom max(|e|,1e-3)) leaves no room for bf16/fp16 transport.
"""

import hashlib

import numpy as np
import jax
import jax.numpy as jnp
from jax import lax

_DN = ("NCHW", "OIHW", "NCHW")
_N_CORES = 8

# (out_ch, kernel, stride) for layers 1..4 (layer 0 is done on the host).
_DEV_STRIDES = (2, 1, 2, 1)

_cache = {"wkey": None, "fn": None}


def _conv0_host(x, W, b):
    """Stride-2 3x3 conv, pad 1, + bias + relu, in fp32 on the host.

    x: [B,C,H,W] -> [B,O,H//2,W//2]
    """
    Bn, C, H, Wd = x.shape
    O = W.shape[0]
    Ho, Wo = H // 2, Wd // 2
    xp = np.zeros((Bn, C, H + 2, Wd + 2), np.float32)
    xp[:, :, 1:-1, 1:-1] = x
    N = Bn * Ho * Wo
    col = np.empty((C * 9, N), np.float32)
    k = 0
    for c in range(C):
        for dy in range(3):
            for dx in range(3):
                np.copyto(
                    col[k].reshape(Bn, Ho, Wo), xp[:, c, dy : dy + H : 2, dx : dx + Wd : 2]
                )
                k += 1
    out = W.reshape(O, C * 9) @ col  # [O, N]
    out += b[:, None]
    np.maximum(out, 0, out=out)
    return np.ascontiguousarray(out.reshape(O, Bn, Ho, Wo).transpose(1, 0, 2, 3))


def _build_fn(Ws, bs):
    """pmap'd L1..L4 + xcorr with the weights closed over as constants."""
    Wj = [jnp.asarray(W) for W in Ws]
    bj = [jnp.asarray(b) for b in bs]

    def shard_fn(f0, c0):
        def enc(x):
            for i, s in enumerate(_DEV_STRIDES):
                x = lax.conv_general_dilated(
                    x, Wj[i], (s, s), ((1, 1), (1, 1)), dimension_numbers=_DN
                )
                x = x + bj[i][None, :, None, None]
                if i < 3:
                    x = jax.nn.relu(x)
            return x

        frame_fm = enc(f0)  # [bl,256,32,32]
        crop_fm = enc(c0)   # [bl,256,8,8]

        def xcorr(f, k):
            return lax.conv_general_dilated(
                f[None], k[None], (1, 1), "VALID", dimension_numbers=_DN
            )[0]

        return jax.vmap(xcorr)(frame_fm, crop_fm)  # [bl,1,25,25]

    return jax.pmap(shard_fn, devices=jax.devices()[:_N_CORES])


def kernel(crop, frame, W0, b0, W1, b1, W2, b2, W3, b3, W4, b4, gamma, beta):
    crop = np.asarray(crop, np.float32)
    frame = np.asarray(frame, np.float32)
    W0 = np.asarray(W0, np.float32)
    b0 = np.asarray(b0, np.float32)
    Ws = [np.asarray(w, np.float32) for w in (W1, W2, W3, W4)]
    bs = [np.asarray(b, np.float32) for b in (b1, b2, b3, b4)]

    # Host layer 0 (the transfer-size reducer).
    f0 = _conv0_host(frame, W0, b0)  # [64,3,128,128]
    c0 = _conv0_host(crop, W0, b0)   # [64,3,32,32]

    B = f0.shape[0]
    bl = B // _N_CORES
    f0_sh = f0.reshape(_N_CORES, bl, *f0.shape[1:])
    c0_sh = c0.reshape(_N_CORES, bl, *c0.shape[1:])

    # Weight-baked executable, rebuilt only if the weight bytes change.
    h = hashlib.blake2b(digest_size=16)
    for a in (*Ws, *bs):
        h.update(a.tobytes())
    wkey = h.hexdigest()
    if _cache["wkey"] != wkey:
        _cache["fn"] = _build_fn(Ws, bs)
        _cache["wkey"] = wkey

    rmap_sh = _cache["fn"](f0_sh, c0_sh)
    rmap = np.asarray(rmap_sh).reshape(B, 1, 25, 25).astype(np.float64)

    # Global BatchNorm2d(1), training mode; exact batch stats on the host.
    mean = rmap.mean()
    var = rmap.var()
    g = np.asarray(gamma, np.float32).reshape(1, -1, 1, 1)
    bt = np.asarray(beta, np.float32).reshape(1, -1, 1, 1)
    out = (rmap - mean) / np.sqrt(var + 1e-5)
    out = out.astype(np.float32) * g + bt
    return out.astype(np.float32)
